# revision 29
# baseline (speedup 1.0000x reference)
"""Trainium2 Bass kernel for a 2-layer transformer encoder (B=8,S=1024,D=512,H=8,DK=12,DV=32,FF=2048).

Sharding: data-parallel over batch - one batch element per NeuronCore, 8 cores,
no collectives. Each core runs the full 2-layer encoder on its (S, D) slice.

v4 (vs v3 @387us, v2 @483us):
- bn_stats for each LN is emitted immediately after the producing
  residual-add, so only aggr+Newton+normalize remain on the tail chain.
- One vectorized quake-rsqrt Newton chain per 4-tile group on [128,4]
  (strided mean/var views, no deinterleave copies).
- Targeted force-drain: attention steps drain the pending tail only up to
  the closure they actually need (kt quad before its scores, v[mk] before
  its ctx); qt and leftovers drip into the post-force exp steps.
- Boot: K/V/Q half-0 projections start after only tiles 0-3 are
  transposed; group-1 transposes interleave with them.  Exp table
  preloaded at t=0.
- bf16 nx + bf16 transposes; ctx-denominator broadcast via 4 concurrent
  K=1 matmuls; scores->exp->ctx with drain-before-scores (trace order ==
  dataflow order: a read traced before its producer write races).
"""

import sys

sys.path.insert(0, "/opt/trn_rl_repo")

import numpy as np
import ml_dtypes

import concourse.bass as bass
import concourse.tile as tile
from concourse import bacc, mybir
from concourse.masks import make_identity

F32 = mybir.dt.float32
BF16 = mybir.dt.bfloat16
I32 = mybir.dt.int32
F8 = mybir.dt.float8e4

L = 2
S = 1024
D = 512
H = 8
DK = 12
DV = 32
FF = 2048
SM = S // 128   # 8 token tiles
DC = D // 128   # 4 D-chunks
FC = FF // 128  # 16 FF-chunks
SCALE = float(1.0 / np.sqrt(np.float32(DK)))
NCORES = 8

AF = mybir.ActivationFunctionType
ALU = mybir.AluOpType


def build_module(with_mask=False):
    nc = bacc.Bacc("TRN2", target_bir_lowering=False, debug=False, num_devices=NCORES)

    x_in = nc.dram_tensor("x", [S, D], F32, kind="ExternalInput")
    wq_d = nc.dram_tensor("wq", [L, DC, 128, 256], BF16, kind="ExternalInput")
    wk_d = nc.dram_tensor("wk", [L, DC, 128, 256], BF16, kind="ExternalInput")
    wv_d = nc.dram_tensor("wv", [L, DC, 128, 256], BF16, kind="ExternalInput")
    wx_d = nc.dram_tensor("wx", [L, 128, 2, D], BF16, kind="ExternalInput")
    w1_d = nc.dram_tensor("w1", [L, DC, 128, FF], BF16, kind="ExternalInput")
    w2_d = nc.dram_tensor("w2", [L, FC, 128, D], BF16, kind="ExternalInput")
    mask_d = None
    if with_mask:
        mask_d = nc.dram_tensor("maskf", [S], F32, kind="ExternalInput")
    out_d = nc.dram_tensor("out", [S, D], F32, kind="ExternalOutput")
    out_ap = out_d.rearrange("(m p) d -> p m d", p=128)

    with tile.TileContext(nc) as tc:
        with (
            tc.tile_pool(name="const", bufs=1) as const,
            tc.tile_pool(name="wts", bufs=2) as wts,
            tc.tile_pool(name="wbig", bufs=2) as wbig,
            tc.tile_pool(name="acts", bufs=1) as acts,
            tc.tile_pool(name="nx1p", bufs=4) as nx1p,
            tc.tile_pool(name="nx2p", bufs=4) as nx2p,
            tc.tile_pool(name="trs", bufs=2) as trs,
            tc.tile_pool(name="trs2", bufs=2) as trs2,
            tc.tile_pool(name="pt", bufs=2) as ptp,
            tc.tile_pool(name="kvp", bufs=2) as kvp,
            tc.tile_pool(name="hT", bufs=1) as htp,
            tc.tile_pool(name="small", bufs=3) as small,
            tc.tile_pool(name="mult", bufs=2) as multp,
            tc.tile_pool(name="norm1", bufs=1) as normp,
            tc.tile_pool(name="ps_sp", bufs=1, space="PSUM") as ps_sp,
            tc.tile_pool(name="ps_cq", bufs=1, space="PSUM") as ps_cq,
            tc.tile_pool(name="ps", bufs=2, space="PSUM") as psb,
        ):
            ident_bf = const.tile([128, 128], BF16)
            make_identity(nc, ident_bf)
            ones = const.tile([128, 1], BF16)
            nc.vector.memset(ones[:], 1.0)
            onesT = const.tile([128, 32], BF16)
            nc.vector.memset(onesT[:], 1.0)
            # preload the exp activation table at t=0 so the first real
            # exp doesn't pay the ~2.7us PSEUDO_LOAD on the critical path
            warm = const.tile([128, 1], F32)
            nc.vector.memset(warm[:], 0.0)
            nc.scalar.activation(out=warm[:], in_=warm[:], func=AF.Exp)

            # residual stream, token-major: x[:, m, :] is tokens 128m..128m+127
            x = acts.tile([128, SM, D], F32, tag="x")
            xsrc = x_in.rearrange("(m p) d -> p m d", p=128)
            for m in range(SM):
                nc.sync.dma_start(out=x[:, m, :], in_=xsrc[:, m, :])

            mask_sb = None
            if with_mask:
                mask_sb = const.tile([128, SM], F32)
                nc.sync.dma_start(
                    out=mask_sb[:], in_=mask_d.rearrange("(m p) -> p m", p=128)
                )

            # per-layer weights (bufs=2 rotates across layers)
            W = []
            for l in range(L):
                wq = wts.tile([128, DC, 256], BF16, tag="wq")
                wk = wts.tile([128, DC, 256], BF16, tag="wk")
                wv = wts.tile([128, DC, 256], BF16, tag="wv")
                wx = wts.tile([128, 2, D], BF16, tag="wx")
                w1 = wbig.tile([128, DC, FF], BF16, tag="w1")
                w2 = wbig.tile([128, FC, D], BF16, tag="w2")
                nc.sync.dma_start(out=wq[:], in_=wq_d[l].rearrange("c p n -> p c n"))
                nc.sync.dma_start(out=wk[:], in_=wk_d[l].rearrange("c p n -> p c n"))
                nc.sync.dma_start(out=wv[:], in_=wv_d[l].rearrange("c p n -> p c n"))
                nc.sync.dma_start(out=wx[:], in_=wx_d[l])
                nc.sync.dma_start(out=w1[:], in_=w1_d[l].rearrange("c p n -> p c n"))
                nc.sync.dma_start(out=w2[:], in_=w2_d[l].rearrange("c p n -> p c n"))
                W.append((wq, wk, wv, wx, w1, w2))

            ctxT = acts.tile([128, 2, S], BF16, tag="ctxT")

            def alloc_kv():
                return dict(
                    qt=kvp.tile([128, 2, S], BF16, tag="qt", name="qt"),
                    kt=kvp.tile([128, 2, S], BF16, tag="kt", name="kt"),
                    v=kvp.tile([128, SM, 256], BF16, tag="v", name="v"),
                )

            def emit_ln_stats(m, grp, i, n):
                """bn_stats for tile m into the group's shared st tile."""
                if "st" not in grp:
                    grp["st"] = small.tile([128, n, 6], F32, tag="st", name="st")
                nc.vector.bn_stats(out=grp["st"][:, i, :], in_=x[:, m, :])

            def emit_ln_finish(ms, grp, nx_tiles, pool):
                """bn_aggr per tile + ONE vectorized quake-rsqrt Newton chain
                on [128, n] (~0.2% max err) + per-tile normalize to bf16."""
                n = len(ms)
                st = grp["st"]
                mv = small.tile([128, n, 2], F32, tag="mv", name="mv")
                for i in range(n):
                    nc.vector.bn_aggr(out=mv[:, i, :], in_=st[:, i, :])
                var = mv[:, :, 1:2]
                mean = mv[:, :, 0:1]
                ti = small.tile([128, n], I32, tag="ti", name="ti")
                rstd = small.tile([128, n], F32, tag="rstd", name="rstd")
                u = small.tile([128, n], F32, tag="u", name="u")
                nc.vector.tensor_scalar(
                    out=ti[:], in0=var.bitcast(I32), scalar1=1, scalar2=None,
                    op0=ALU.logical_shift_right,
                )
                nc.vector.tensor_scalar(
                    out=rstd[:].bitcast(I32), in0=ti[:], scalar1=-1,
                    scalar2=0x5F3759DF, op0=ALU.mult, op1=ALU.add,
                )
                nc.vector.tensor_tensor(out=u[:], in0=rstd[:], in1=rstd[:], op=ALU.mult)
                nc.vector.tensor_tensor(out=u[:], in0=u[:], in1=var, op=ALU.mult)
                nc.vector.tensor_scalar(
                    out=u[:], in0=u[:], scalar1=-0.5 * float(D) / (D - 1), scalar2=1.5,
                    op0=ALU.mult, op1=ALU.add,
                )
                nc.vector.tensor_tensor(out=rstd[:], in0=rstd[:], in1=u[:], op=ALU.mult)
                nmr = small.tile([128, n], F32, tag="nmr", name="nmr")
                nc.vector.scalar_tensor_tensor(
                    out=nmr[:], in0=mean, scalar=-1.0, in1=rstd[:],
                    op0=ALU.mult, op1=ALU.mult,
                )
                for i, m in enumerate(ms):
                    nx = pool.tile([128, D], BF16, tag="nx", name="nx")
                    nc.vector.tensor_scalar(
                        out=nx[:], in0=x[:, m, :], scalar1=rstd[:, i:i + 1],
                        scalar2=nmr[:, i:i + 1], op0=ALU.mult, op1=ALU.add,
                    )
                    nx_tiles[m] = nx

            def emit_transp_m(nx_tiles, nT, m, moff=0):
                """bf16 PE transpose of tile m -> nT[:, :, 128(m-moff)..]."""
                nx = nx_tiles.pop(m)
                tp = psb.tile([128, 512], BF16, tag="ps", name="tp")
                for c in range(DC):
                    nc.tensor.transpose(
                        tp[:, 128 * c:128 * (c + 1)], nx[:, 128 * c:128 * (c + 1)],
                        ident_bf[:],
                    )
                lm = m - moff
                nc.vector.tensor_copy(
                    out=nT[:, :, 128 * lm:128 * (lm + 1)],
                    in_=tp[:].rearrange("p (c t) -> p c t", c=DC),
                )

            def emit_qk_proj(kv, key, w, nT, quad, th, pair=None):
                dst = kv[key]
                if pair is None:
                    lo, n = 512 * th, 512
                else:
                    lo, n = 512 * th + 256 * pair, 256
                pp = psb.tile([128, 512], F32, tag="ps", name="pp")
                for c in range(DC):
                    nc.tensor.matmul(
                        pp[:, 0:n], w[:, c, 128 * quad:128 * (quad + 1)],
                        nT[:, c, lo:lo + n],
                        start=(c == 0), stop=(c == DC - 1),
                    )
                nc.scalar.copy(out=dst[:, quad, lo:lo + n], in_=pp[:, 0:n])

            def emit_v_proj_m(kv, nT, wv, m):
                v = kv["v"]
                pp = psb.tile([128, 512], F32, tag="ps", name="pp")
                for c in range(DC):
                    nc.tensor.matmul(
                        pp[:, 0:256],
                        nT[:, c, 128 * m:128 * (m + 1)],
                        wv[:, c, :],
                        start=(c == 0), stop=(c == DC - 1),
                    )
                nc.scalar.copy(out=v[:, m, :], in_=pp[:, 0:256])

            def emit_kvq_pair(kv, nT, wq, wk, wv, th, pair, stage):
                """Projection closures for token-pair `pair` of half th."""
                cl = []
                for quad in range(2):
                    cl.append((480, ("kt", quad, pair), stage, emit_qk_proj,
                               (kv, "kt", wk, nT, quad, th, pair)))
                for m in range(4 * th + 2 * pair, 4 * th + 2 * pair + 2):
                    cl.append((480, ("v", m), stage, emit_v_proj_m, (kv, nT, wv, m)))
                for quad in range(2):
                    cl.append((480, ("qt", quad, pair), stage, emit_qk_proj,
                               (kv, "qt", wq, nT, quad, th, pair)))
                return cl

            def emit_scores_exp(kv, quad, mk, qh):
                """4 row-tiled concurrent score MMs + one exp -> pt tile."""
                kt, qt = kv["kt"], kv["qt"]
                sp = ps_sp.tile([128, 4, 512], F32, tag="sp", name="sp")
                for j in range(4):
                    nc.tensor.matmul(
                        sp[:, j, :],
                        kt[32 * j:32 * j + 32, quad, 128 * mk:128 * (mk + 1)],
                        qt[32 * j:32 * j + 32, quad, 512 * qh:512 * (qh + 1)],
                        start=True, stop=True,
                        tile_position=(32 * j, 0),
                    )
                pt = ptp.tile([128, 4, 512], BF16, tag="pt", name="pt")
                nc.scalar.activation(out=pt[:], in_=sp[:], func=AF.Exp, scale=SCALE)
                if with_mask:
                    nc.vector.tensor_scalar_mul(
                        out=pt[:], in0=pt[:], scalar1=mask_sb[:, mk:mk + 1]
                    )
                return pt

            def emit_ctx(kv, quad, mk, pt, cq):
                """4 col-tiled ctx MMs + 4 col-tiled denominator MMs."""
                v = kv["v"]
                for j in range(4):
                    h = 4 * quad + j
                    nc.tensor.matmul(
                        cq[32 * j:32 * j + 32, 0, :],
                        v[:, mk, 32 * h:32 * h + 32],
                        pt[:, j, :],
                        start=(mk == 0), stop=(mk == SM - 1),
                        tile_position=(0, 32 * j),
                    )
                for j in range(4):
                    nc.tensor.matmul(
                        cq[32 * j:32 * j + 1, 1, :],
                        ones[:],
                        pt[:, j, :],
                        start=(mk == 0), stop=(mk == SM - 1),
                        tile_position=(0, 32 * j),
                    )

            def emit_ctx_norm(quad, qh, cq):
                # Evacuate ctx + denominators, broadcast the 4 denominator
                # rows across their 32-row bands with 4 concurrent K=1
                # matmuls (ones outer product), then normalize on DVE.
                cqc = multp.tile([128, 512], F32, tag="cqc", name="cqc")
                nc.vector.tensor_copy(out=cqc[:], in_=cq[:, 0, :])
                den97 = normp.tile([97, 512], BF16, tag="den97", name="den97")
                nc.vector.tensor_copy(out=den97[:], in_=cq[0:97, 1, :])
                bc = psb.tile([128, 512], F32, tag="ps", name="bc")
                for j in range(4):
                    nc.tensor.matmul(
                        bc[32 * j:32 * j + 32, :],
                        onesT[32 * j:32 * j + 1, :],
                        den97[32 * j:32 * j + 1, :],
                        start=True, stop=True,
                        tile_position=(32 * j, 32 * j),
                    )
                rec = normp.tile([128, 512], F32, tag="rec", name="rec")
                nc.vector.reciprocal_approx_fast(out=rec[:], in_=bc[:])
                nc.vector.scalar_tensor_tensor(
                    out=ctxT[:, quad, 512 * qh:512 * (qh + 1)],
                    in0=cqc[:], scalar=1.0, in1=rec[:],
                    op0=ALU.mult, op1=ALU.mult,
                )

            def emit_outproj(m, wx, grp, i, n):
                ap_ = psb.tile([128, 512], F32, tag="ps", name="ap_")
                for quad in range(2):
                    nc.tensor.matmul(
                        ap_[:],
                        ctxT[:, quad, 128 * m:128 * (m + 1)],
                        wx[:, quad, :],
                        start=(quad == 0), stop=(quad == 1),
                    )
                nc.vector.tensor_add(out=x[:, m, :], in0=ap_[:], in1=x[:, m, :])
                emit_ln_stats(m, grp, i, n)

            def emit_ffn1(ff, w1, n2T, hT):
                hp = psb.tile([128, 512], F32, tag="ps", name="hp")
                for c in range(DC):
                    nc.tensor.matmul(
                        hp[:], w1[:, c, 128 * ff:128 * (ff + 1)],
                        n2T[:, c, :],
                        start=(c == 0), stop=(c == DC - 1),
                    )
                nc.vector.tensor_scalar_max(
                    out=hT[:, ff, :], in0=hp[:], scalar1=0.0
                )

            def emit_ffn2(m, qh, w2, hT, l, grp, i, n):
                lm = m - 4 * qh
                yp = psb.tile([128, 512], F32, tag="ps", name="yp")
                for ff in range(FC):
                    nc.tensor.matmul(
                        yp[:], hT[:, ff, 128 * lm:128 * (lm + 1)], w2[:, ff, :],
                        start=(ff == 0), stop=(ff == FC - 1),
                    )
                nc.vector.tensor_add(out=x[:, m, :], in0=yp[:], in1=x[:, m, :])
                if l == L - 1:
                    nc.sync.dma_start(out=out_ap[:, m, :], in_=x[:, m, :])
                else:
                    emit_ln_stats(m, grp, i, n)

            # ---------------- program ----------------
            nTs = {}
            KV = {}

            def make_tail(l, qh):
                """Engine-phase-grouped closure list for token-half qh with
                pair-split LN1'/transposes/projections (v5 ordering)."""
                wq_, wk_, wv_, wx_, w1_, w2_ = W[l]
                ms = list(range(4 * qh, 4 * qh + 4))
                cl = []
                n2T = trs2.tile([128, DC, 512], BF16, tag="n2T", name="n2T")
                hTq = htp.tile([128, FC, 512], BF16, tag="hT", name="hT")
                g2 = {}
                nx2 = {}
                for i, m in enumerate(ms):
                    cl.append((480, None, 0, emit_outproj, (m, wx_, g2, i, len(ms))))
                cl.append((0, None, 0, emit_ln_finish, (ms, g2, nx2, nx2p)))
                for m in ms:
                    cl.append((560, None, 1, emit_transp_m, (nx2, n2T, m, 4 * qh)))
                for ff in range(FC):
                    cl.append((880, None, 2, emit_ffn1, (ff, w1_, n2T, hTq)))
                g1a, g1b = {}, {}
                nx1a, nx1b = {}, {}
                ma, mb = ms[0:2], ms[2:4]
                cl.append((3500, None, 3, emit_ffn2,
                           (ma[0], qh, w2_, hTq, l, g1a, 0, 2)))
                cl.append((3500, None, 3, emit_ffn2,
                           (ma[1], qh, w2_, hTq, l, g1a, 1, 2)))
                if l < L - 1:
                    nTn = nTs[l + 1]
                    wqn, wkn, wvn = W[l + 1][0], W[l + 1][1], W[l + 1][2]
                    if qh == 0:
                        KV[l + 1] = alloc_kv()
                    cl.append((0, None, 3, emit_ln_finish, (ma, g1a, nx1a, nx1p)))
                    for m in ma:
                        cl.append((560, None, 4, emit_transp_m, (nx1a, nTn, m, 0)))
                    cl.append((3500, None, 4, emit_ffn2,
                               (mb[0], qh, w2_, hTq, l, g1b, 0, 2)))
                    cl.append((3500, None, 4, emit_ffn2,
                               (mb[1], qh, w2_, hTq, l, g1b, 1, 2)))
                    cl.append((0, None, 4, emit_ln_finish, (mb, g1b, nx1b, nx1p)))
                    cl += emit_kvq_pair(KV[l + 1], nTn, wqn, wkn, wvn, qh, 0, 5)
                    for m in mb:
                        cl.append((560, None, 5, emit_transp_m, (nx1b, nTn, m, 0)))
                    cl += emit_kvq_pair(KV[l + 1], nTn, wqn, wkn, wvn, qh, 1, 6)
                else:
                    cl.append((3500, None, 4, emit_ffn2,
                               (mb[0], qh, w2_, hTq, l, g1b, 0, 2)))
                    cl.append((3500, None, 4, emit_ffn2,
                               (mb[1], qh, w2_, hTq, l, g1b, 1, 2)))
                return cl

            def attn_loop(l, qh, pending):
                kv = KV[l]
                total = sum(c for c, k, s, f, a in pending)
                keyidx = {k: i for i, (c, k, s, f, a) in enumerate(pending) if k}
                state = {"done": 0, "spent": 0}

                def drain(need_idx, want_cost, stage_limit=False):
                    crossed = 0
                    while state["done"] < len(pending):
                        c, k, s, f, a = pending[state["done"]]
                        forced = state["done"] < need_idx
                        if not forced:
                            if state["spent"] >= want_cost:
                                break
                            if stage_limit and state["done"] > 0:
                                ps_ = pending[state["done"] - 1][2]
                                if s > ps_:
                                    crossed += 1
                                    if crossed > 1:
                                        break
                        f(*a)
                        state["spent"] += c
                        state["done"] += 1

                prev = None
                for quad in range(2):
                    cq = ps_cq.tile([128, 2, 512], F32, tag="cq", name="cq")
                    for mk in range(SM):
                        step = quad * SM + mk + 1   # 1..16
                        need = 0
                        if qh == 0:
                            if mk >= 4:
                                need = max(need, keyidx.get(
                                    ("kt", quad, (mk - 4) // 2), -1) + 1)
                            if prev is not None and prev[1] >= 4:
                                need = max(need, keyidx.get(("v", prev[1]), -1) + 1)
                        want = (total * max(0, step - 1)) // 24
                        drain(need, want, stage_limit=False)
                        pt = emit_scores_exp(kv, quad, mk, qh)
                        if prev is not None:
                            pq, pm, ppt, pcq = prev
                            emit_ctx(kv, pq, pm, ppt, pcq)
                            if pm == SM - 1:
                                emit_ctx_norm(pq, qh, pcq)
                        prev = (quad, mk, pt, cq)
                need = len(pending)
                if qh == 0 and prev is not None and prev[1] >= 4:
                    need = keyidx.get(("v", prev[1]), len(pending) - 1) + 1
                    drain(need, 0)
                pq, pm, ppt, pcq = prev
                emit_ctx(kv, pq, pm, ppt, pcq)
                emit_ctx_norm(pq, qh, pcq)
                drain(len(pending), total)

            # boot: LN group 0 + transposes 0-3, then K/V/Q half-0
            # projections interleaved with group 1 + transposes 4-7.
            for l in range(L):
                nTs[l] = trs.tile([128, DC, S], BF16, tag="nT", name="nT")
            wq0, wk0, wv0 = W[0][0], W[0][1], W[0][2]
            KV[0] = alloc_kv()
            nxb = {}
            queue = []
            for pair in range(4):
                gb = {}
                for i in range(2):
                    emit_ln_stats(2 * pair + i, gb, i, 2)
                emit_ln_finish([2 * pair, 2 * pair + 1], gb, nxb, nx1p)
                emit_transp_m(nxb, nTs[0], 2 * pair, 0)
                emit_transp_m(nxb, nTs[0], 2 * pair + 1, 0)
                for q in queue:
                    q[3](*q[4])
                queue = []
                if pair == 0:
                    queue = emit_kvq_pair(KV[0], nTs[0], wq0, wk0, wv0, 0, 0, 0)
                elif pair == 1:
                    queue = emit_kvq_pair(KV[0], nTs[0], wq0, wk0, wv0, 0, 1, 0)
            for q in queue:
                q[3](*q[4])

            pending = (emit_kvq_pair(KV[0], nTs[0], wq0, wk0, wv0, 1, 0, 0)
                       + emit_kvq_pair(KV[0], nTs[0], wq0, wk0, wv0, 1, 1, 1))
            for l in range(L):
                attn_loop(l, 0, pending)
                pending = make_tail(l, 0)
                attn_loop(l, 1, pending)
                pending = make_tail(l, 1)
            # final tail (layer L-1 half 1) runs serially; its FFN2 closures
            # stream the output DMAs per token tile.
            for _, _, _, fn, args in pending:
                fn(*args)

    nc.compile()
    return nc


_CACHE = {}


def _get_module(with_mask):
    key = (with_mask,)
    if key not in _CACHE:
        _CACHE[key] = build_module(with_mask=with_mask)
    return _CACHE[key]


def _prep_weights(Wq, Wk, Wv, Wx, W1, W2):
    bf = ml_dtypes.bfloat16

    # Q/K: pad head columns from 12 to 32 (head h=4q+j at col 128q+32j)
    def pad_qk(w):  # [L, 512, 96] -> [L, DC, 128, 256]
        out = np.zeros((L, D, 256), np.float32)
        for h in range(H):
            q, j = divmod(h, 4)
            out[:, :, 128 * q + 32 * j:128 * q + 32 * j + DK] = (
                w[:, :, DK * h:DK * (h + 1)]
            )
        return np.ascontiguousarray(out.reshape(L, DC, 128, 256)).astype(bf)

    wq = pad_qk(np.asarray(Wq))
    wk = pad_qk(np.asarray(Wk))
    wv = np.ascontiguousarray(np.asarray(Wv).reshape(L, DC, 128, 256)).astype(bf)
    # Wx rows (h=4q+j, dd) -> [32j+dd, quad, :]
    wx = np.ascontiguousarray(
        np.asarray(Wx).reshape(L, 2, 4, 32, D).transpose(0, 2, 3, 1, 4)
        .reshape(L, 128, 2, D)
    ).astype(bf)
    w1 = np.ascontiguousarray(np.asarray(W1).reshape(L, DC, 128, FF)).astype(bf)
    w2 = np.ascontiguousarray(np.asarray(W2).reshape(L, FC, 128, D)).astype(bf)
    return dict(wq=wq, wk=wk, wv=wv, wx=wx, w1=w1, w2=w2)


def kernel(inputs, mask, Wq, bq, Wk, bk, Wv, bv, Wx, bx, W1, b1, W2, b2, gamma, beta):
    inputs = np.asarray(inputs, np.float32)
    mask = np.asarray(mask)
    for nm, b in (("bq", bq), ("bk", bk), ("bv", bv), ("bx", bx), ("b1", b1), ("b2", b2)):
        assert not np.any(np.asarray(b)), f"nonzero bias {nm} not supported"
    assert np.all(np.asarray(gamma) == 1.0) and not np.any(np.asarray(beta)), (
        "non-identity layernorm affine not supported"
    )

    with_mask = bool(np.any(np.asarray(mask) == 0))
    nc = _get_module(with_mask)
    wmap = _prep_weights(
        np.asarray(Wq, np.float32), np.asarray(Wk, np.float32),
        np.asarray(Wv, np.float32), np.asarray(Wx, np.float32),
        np.asarray(W1, np.float32), np.asarray(W2, np.float32),
    )

    in_maps = []
    for b in range(NCORES):
        m = dict(wmap)
        m["x"] = np.ascontiguousarray(inputs[b])
        if with_mask:
            m["maskf"] = np.ascontiguousarray((mask[b, 0] != 0).astype(np.float32))
        in_maps.append(m)

    import os
    from concourse.bass_utils import run_bass_kernel_spmd

    kw = {}
    tdir = os.environ.get("BASS_KERNEL_TRACE_DIR")
    if tdir:
        kw = dict(trace=True, tmpdir=tdir)
    res = run_bass_kernel_spmd(nc, in_maps, core_ids=list(range(NCORES)), **kw)
    global LAST_EXEC_NS
    LAST_EXEC_NS = res.exec_time_ns
    out = np.stack([res.results[i]["out"] for i in range(NCORES)], axis=0)
    return out.astype(np.float32)


LAST_EXEC_NS = None


# revision 30
# speedup vs baseline: 1.0497x; 1.0497x over previous
"""Trainium2 Bass kernel for a 2-layer transformer encoder (B=8,S=1024,D=512,H=8,DK=12,DV=32,FF=2048).

Sharding: data-parallel over batch - one batch element per NeuronCore, 8 cores,
no collectives. Each core runs the full 2-layer encoder on its (S, D) slice.

v4 (vs v3 @387us, v2 @483us):
- bn_stats for each LN is emitted immediately after the producing
  residual-add, so only aggr+Newton+normalize remain on the tail chain.
- One vectorized quake-rsqrt Newton chain per 4-tile group on [128,4]
  (strided mean/var views, no deinterleave copies).
- Targeted force-drain: attention steps drain the pending tail only up to
  the closure they actually need (kt quad before its scores, v[mk] before
  its ctx); qt and leftovers drip into the post-force exp steps.
- Boot: K/V/Q half-0 projections start after only tiles 0-3 are
  transposed; group-1 transposes interleave with them.  Exp table
  preloaded at t=0.
- bf16 nx + bf16 transposes; ctx-denominator broadcast via 4 concurrent
  K=1 matmuls; scores->exp->ctx with drain-before-scores (trace order ==
  dataflow order: a read traced before its producer write races).
"""

import sys

sys.path.insert(0, "/opt/trn_rl_repo")

import numpy as np
import ml_dtypes

import concourse.bass as bass
import concourse.tile as tile
from concourse import bacc, mybir
from concourse.masks import make_identity

F32 = mybir.dt.float32
BF16 = mybir.dt.bfloat16
I32 = mybir.dt.int32
F8 = mybir.dt.float8e4

L = 2
S = 1024
D = 512
H = 8
DK = 12
DV = 32
FF = 2048
SM = S // 128   # 8 token tiles
DC = D // 128   # 4 D-chunks
FC = FF // 128  # 16 FF-chunks
SCALE = float(1.0 / np.sqrt(np.float32(DK)))
NCORES = 8

AF = mybir.ActivationFunctionType
ALU = mybir.AluOpType


def build_module(with_mask=False):
    nc = bacc.Bacc("TRN2", target_bir_lowering=False, debug=False, num_devices=NCORES)

    x_in = nc.dram_tensor("x", [S, D], F32, kind="ExternalInput")
    wq_d = nc.dram_tensor("wq", [L, DC, 128, 256], BF16, kind="ExternalInput")
    wk_d = nc.dram_tensor("wk", [L, DC, 128, 256], BF16, kind="ExternalInput")
    wv_d = nc.dram_tensor("wv", [L, DC, 128, 256], BF16, kind="ExternalInput")
    wx_d = nc.dram_tensor("wx", [L, 128, 2, D], BF16, kind="ExternalInput")
    w1_d = nc.dram_tensor("w1", [L, DC, 128, FF], BF16, kind="ExternalInput")
    w2_d = nc.dram_tensor("w2", [L, FC, 128, D], BF16, kind="ExternalInput")
    mask_d = None
    if with_mask:
        mask_d = nc.dram_tensor("maskf", [S], F32, kind="ExternalInput")
    out_d = nc.dram_tensor("out", [S, D], F32, kind="ExternalOutput")
    out_ap = out_d.rearrange("(m p) d -> p m d", p=128)

    with tile.TileContext(nc) as tc:
        with (
            tc.tile_pool(name="const", bufs=1) as const,
            tc.tile_pool(name="wts", bufs=2) as wts,
            tc.tile_pool(name="wbig", bufs=2) as wbig,
            tc.tile_pool(name="acts", bufs=1) as acts,
            tc.tile_pool(name="nx1p", bufs=4) as nx1p,
            tc.tile_pool(name="nx2p", bufs=4) as nx2p,
            tc.tile_pool(name="trs", bufs=2) as trs,
            tc.tile_pool(name="trs2", bufs=2) as trs2,
            tc.tile_pool(name="pt", bufs=2) as ptp,
            tc.tile_pool(name="kvp", bufs=2) as kvp,
            tc.tile_pool(name="hT", bufs=1) as htp,
            tc.tile_pool(name="small", bufs=3) as small,
            tc.tile_pool(name="mult", bufs=2) as multp,
            tc.tile_pool(name="norm1", bufs=1) as normp,
            tc.tile_pool(name="ps_sp", bufs=1, space="PSUM") as ps_sp,
            tc.tile_pool(name="ps_cq", bufs=1, space="PSUM") as ps_cq,
            tc.tile_pool(name="ps", bufs=2, space="PSUM") as psb,
        ):
            ident_bf = const.tile([128, 128], BF16)
            make_identity(nc, ident_bf)
            ones = const.tile([128, 1], BF16)
            nc.vector.memset(ones[:], 1.0)
            onesT = const.tile([128, 32], BF16)
            nc.vector.memset(onesT[:], 1.0)
            # preload the exp activation table at t=0 so the first real
            # exp doesn't pay the ~2.7us PSEUDO_LOAD on the critical path
            warm = const.tile([128, 1], F32)
            nc.vector.memset(warm[:], 0.0)
            nc.scalar.activation(out=warm[:], in_=warm[:], func=AF.Exp)

            # residual stream, token-major: x[:, m, :] is tokens 128m..128m+127
            x = acts.tile([128, SM, D], F32, tag="x")
            xsrc = x_in.rearrange("(m p) d -> p m d", p=128)
            for m in range(SM):
                nc.sync.dma_start(out=x[:, m, :], in_=xsrc[:, m, :])

            mask_sb = None
            if with_mask:
                mask_sb = const.tile([128, SM], F32)
                nc.sync.dma_start(
                    out=mask_sb[:], in_=mask_d.rearrange("(m p) -> p m", p=128)
                )

            # per-layer weights (bufs=2 rotates across layers)
            W = []
            for l in range(L):
                wq = wts.tile([128, DC, 256], BF16, tag="wq")
                wk = wts.tile([128, DC, 256], BF16, tag="wk")
                wv = wts.tile([128, DC, 256], BF16, tag="wv")
                wx = wts.tile([128, 2, D], BF16, tag="wx")
                w1 = wbig.tile([128, DC, FF], BF16, tag="w1")
                w2 = wbig.tile([128, FC, D], BF16, tag="w2")
                nc.sync.dma_start(out=wq[:], in_=wq_d[l].rearrange("c p n -> p c n"))
                nc.sync.dma_start(out=wk[:], in_=wk_d[l].rearrange("c p n -> p c n"))
                nc.sync.dma_start(out=wv[:], in_=wv_d[l].rearrange("c p n -> p c n"))
                nc.sync.dma_start(out=wx[:], in_=wx_d[l])
                nc.sync.dma_start(out=w1[:], in_=w1_d[l].rearrange("c p n -> p c n"))
                nc.sync.dma_start(out=w2[:], in_=w2_d[l].rearrange("c p n -> p c n"))
                W.append((wq, wk, wv, wx, w1, w2))

            ctxT = acts.tile([128, 2, S], BF16, tag="ctxT")

            def alloc_kv():
                return dict(
                    qt=kvp.tile([128, 2, S], BF16, tag="qt", name="qt"),
                    kt=kvp.tile([128, 2, S], BF16, tag="kt", name="kt"),
                    v=kvp.tile([128, SM, 256], BF16, tag="v", name="v"),
                )

            def emit_ln_stats(m, grp, i, n):
                """bn_stats for tile m into the group's shared st tile."""
                if "st" not in grp:
                    grp["st"] = small.tile([128, n, 6], F32, tag="st", name="st")
                nc.vector.bn_stats(out=grp["st"][:, i, :], in_=x[:, m, :])

            def emit_ln_finish(ms, grp, nx_tiles, pool):
                """bn_aggr per tile + ONE vectorized quake-rsqrt Newton chain
                on [128, n] (~0.2% max err) + per-tile normalize to bf16."""
                n = len(ms)
                st = grp["st"]
                mv = small.tile([128, n, 2], F32, tag="mv", name="mv")
                for i in range(n):
                    nc.vector.bn_aggr(out=mv[:, i, :], in_=st[:, i, :])
                var = mv[:, :, 1:2]
                mean = mv[:, :, 0:1]
                ti = small.tile([128, n], I32, tag="ti", name="ti")
                rstd = small.tile([128, n], F32, tag="rstd", name="rstd")
                u = small.tile([128, n], F32, tag="u", name="u")
                nc.vector.tensor_scalar(
                    out=ti[:], in0=var.bitcast(I32), scalar1=1, scalar2=None,
                    op0=ALU.logical_shift_right,
                )
                nc.vector.tensor_scalar(
                    out=rstd[:].bitcast(I32), in0=ti[:], scalar1=-1,
                    scalar2=0x5F3759DF, op0=ALU.mult, op1=ALU.add,
                )
                nc.vector.tensor_tensor(out=u[:], in0=rstd[:], in1=rstd[:], op=ALU.mult)
                nc.vector.tensor_tensor(out=u[:], in0=u[:], in1=var, op=ALU.mult)
                nc.vector.tensor_scalar(
                    out=u[:], in0=u[:], scalar1=-0.5 * float(D) / (D - 1), scalar2=1.5,
                    op0=ALU.mult, op1=ALU.add,
                )
                nc.vector.tensor_tensor(out=rstd[:], in0=rstd[:], in1=u[:], op=ALU.mult)
                nmr = small.tile([128, n], F32, tag="nmr", name="nmr")
                nc.vector.scalar_tensor_tensor(
                    out=nmr[:], in0=mean, scalar=-1.0, in1=rstd[:],
                    op0=ALU.mult, op1=ALU.mult,
                )
                for i, m in enumerate(ms):
                    nx = pool.tile([128, D], BF16, tag="nx", name="nx")
                    nc.vector.tensor_scalar(
                        out=nx[:], in0=x[:, m, :], scalar1=rstd[:, i:i + 1],
                        scalar2=nmr[:, i:i + 1], op0=ALU.mult, op1=ALU.add,
                    )
                    nx_tiles[m] = nx

            def emit_transp_m(nx_tiles, nT, m, moff=0):
                """bf16 PE transpose of tile m -> nT[:, :, 128(m-moff)..]."""
                nx = nx_tiles.pop(m)
                tp = psb.tile([128, 512], BF16, tag="ps", name="tp")
                for c in range(DC):
                    nc.tensor.transpose(
                        tp[:, 128 * c:128 * (c + 1)], nx[:, 128 * c:128 * (c + 1)],
                        ident_bf[:],
                    )
                lm = m - moff
                nc.vector.tensor_copy(
                    out=nT[:, :, 128 * lm:128 * (lm + 1)],
                    in_=tp[:].rearrange("p (c t) -> p c t", c=DC),
                )

            def emit_qk_proj(kv, key, w, nT, quad, th, pair=None):
                dst = kv[key]
                if pair is None:
                    lo, n = 512 * th, 512
                else:
                    lo, n = 512 * th + 256 * pair, 256
                pp = psb.tile([128, 512], F32, tag="ps", name="pp")
                for c in range(DC):
                    nc.tensor.matmul(
                        pp[:, 0:n], w[:, c, 128 * quad:128 * (quad + 1)],
                        nT[:, c, lo:lo + n],
                        start=(c == 0), stop=(c == DC - 1),
                    )
                nc.scalar.copy(out=dst[:, quad, lo:lo + n], in_=pp[:, 0:n])

            def emit_v_proj_m(kv, nT, wv, m):
                v = kv["v"]
                pp = psb.tile([128, 512], F32, tag="ps", name="pp")
                for c in range(DC):
                    nc.tensor.matmul(
                        pp[:, 0:256],
                        nT[:, c, 128 * m:128 * (m + 1)],
                        wv[:, c, :],
                        start=(c == 0), stop=(c == DC - 1),
                    )
                nc.scalar.copy(out=v[:, m, :], in_=pp[:, 0:256])

            def emit_kvq_pair(kv, nT, wq, wk, wv, th, pair, stage):
                """Projection closures for token-pair `pair` of half th."""
                cl = []
                for quad in range(2):
                    cl.append((480, ("kt", quad, pair), stage, emit_qk_proj,
                               (kv, "kt", wk, nT, quad, th, pair)))
                for m in range(4 * th + 2 * pair, 4 * th + 2 * pair + 2):
                    cl.append((480, ("v", m), stage, emit_v_proj_m, (kv, nT, wv, m)))
                for quad in range(2):
                    cl.append((480, ("qt", quad, pair), stage, emit_qk_proj,
                               (kv, "qt", wq, nT, quad, th, pair)))
                return cl

            def emit_scores_exp(kv, quad, mk, qh):
                """4 row-tiled concurrent score MMs + one exp -> pt tile."""
                kt, qt = kv["kt"], kv["qt"]
                sp = ps_sp.tile([128, 4, 512], F32, tag="sp", name="sp")
                for j in range(4):
                    nc.tensor.matmul(
                        sp[:, j, :],
                        kt[32 * j:32 * j + 32, quad, 128 * mk:128 * (mk + 1)],
                        qt[32 * j:32 * j + 32, quad, 512 * qh:512 * (qh + 1)],
                        start=True, stop=True,
                        tile_position=(32 * j, 0),
                    )
                pt = ptp.tile([128, 4, 512], BF16, tag="pt", name="pt")
                nc.scalar.activation(out=pt[:], in_=sp[:], func=AF.Exp, scale=SCALE)
                if with_mask:
                    nc.vector.tensor_scalar_mul(
                        out=pt[:], in0=pt[:], scalar1=mask_sb[:, mk:mk + 1]
                    )
                return pt

            def emit_ctx(kv, quad, mk, pt, cq):
                """4 col-tiled ctx MMs + 4 col-tiled denominator MMs."""
                v = kv["v"]
                for j in range(4):
                    h = 4 * quad + j
                    nc.tensor.matmul(
                        cq[32 * j:32 * j + 32, 0, :],
                        v[:, mk, 32 * h:32 * h + 32],
                        pt[:, j, :],
                        start=(mk == 0), stop=(mk == SM - 1),
                        tile_position=(0, 32 * j),
                    )
                for j in range(4):
                    nc.tensor.matmul(
                        cq[32 * j:32 * j + 1, 1, :],
                        ones[:],
                        pt[:, j, :],
                        start=(mk == 0), stop=(mk == SM - 1),
                        tile_position=(0, 32 * j),
                    )

            def emit_ctx_norm(quad, qh, cq):
                # Evacuate ctx + denominators, broadcast the 4 denominator
                # rows across their 32-row bands with 4 concurrent K=1
                # matmuls (ones outer product), then normalize on DVE.
                cqc = multp.tile([128, 512], F32, tag="cqc", name="cqc")
                nc.vector.tensor_copy(out=cqc[:], in_=cq[:, 0, :])
                den97 = normp.tile([97, 512], BF16, tag="den97", name="den97")
                nc.vector.tensor_copy(out=den97[:], in_=cq[0:97, 1, :])
                bc = psb.tile([128, 512], F32, tag="ps", name="bc")
                for j in range(4):
                    nc.tensor.matmul(
                        bc[32 * j:32 * j + 32, :],
                        onesT[32 * j:32 * j + 1, :],
                        den97[32 * j:32 * j + 1, :],
                        start=True, stop=True,
                        tile_position=(32 * j, 32 * j),
                    )
                rec = normp.tile([128, 512], F32, tag="rec", name="rec")
                nc.vector.reciprocal_approx_fast(out=rec[:], in_=bc[:])
                nc.vector.scalar_tensor_tensor(
                    out=ctxT[:, quad, 512 * qh:512 * (qh + 1)],
                    in0=cqc[:], scalar=1.0, in1=rec[:],
                    op0=ALU.mult, op1=ALU.mult,
                )

            def emit_outproj(m, wx, grp, i, n):
                ap_ = psb.tile([128, 512], F32, tag="ps", name="ap_")
                for quad in range(2):
                    nc.tensor.matmul(
                        ap_[:],
                        ctxT[:, quad, 128 * m:128 * (m + 1)],
                        wx[:, quad, :],
                        start=(quad == 0), stop=(quad == 1),
                    )
                nc.vector.tensor_add(out=x[:, m, :], in0=ap_[:], in1=x[:, m, :])
                emit_ln_stats(m, grp, i, n)

            def emit_ffn1(ff, w1, n2T, hT):
                hp = psb.tile([128, 512], F32, tag="ps", name="hp")
                for c in range(DC):
                    nc.tensor.matmul(
                        hp[:], w1[:, c, 128 * ff:128 * (ff + 1)],
                        n2T[:, c, :],
                        start=(c == 0), stop=(c == DC - 1),
                    )
                nc.vector.tensor_scalar_max(
                    out=hT[:, ff, :], in0=hp[:], scalar1=0.0
                )

            def emit_ffn2(m, qh, w2, hT, l, grp, i, n):
                lm = m - 4 * qh
                yp = psb.tile([128, 512], F32, tag="ps", name="yp")
                for ff in range(FC):
                    nc.tensor.matmul(
                        yp[:], hT[:, ff, 128 * lm:128 * (lm + 1)], w2[:, ff, :],
                        start=(ff == 0), stop=(ff == FC - 1),
                    )
                nc.vector.tensor_add(out=x[:, m, :], in0=yp[:], in1=x[:, m, :])
                if l == L - 1:
                    nc.sync.dma_start(out=out_ap[:, m, :], in_=x[:, m, :])
                else:
                    emit_ln_stats(m, grp, i, n)

            # ---------------- program ----------------
            nTs = {}
            KV = {}

            def make_tail(l, qh):
                """Engine-phase-grouped closure list for token-half qh with
                pair-split LN1'/transposes/projections (v5 ordering)."""
                wq_, wk_, wv_, wx_, w1_, w2_ = W[l]
                ms = list(range(4 * qh, 4 * qh + 4))
                cl = []
                n2T = trs2.tile([128, DC, 512], BF16, tag="n2T", name="n2T")
                hTq = htp.tile([128, FC, 512], BF16, tag="hT", name="hT")
                g2 = {}
                nx2 = {}
                for i, m in enumerate(ms):
                    cl.append((480, None, 0, emit_outproj, (m, wx_, g2, i, len(ms))))
                cl.append((0, None, 0, emit_ln_finish, (ms, g2, nx2, nx2p)))
                for m in ms:
                    cl.append((560, None, 1, emit_transp_m, (nx2, n2T, m, 4 * qh)))
                for ff in range(FC):
                    cl.append((880, None, 2, emit_ffn1, (ff, w1_, n2T, hTq)))
                g1a, g1b = {}, {}
                nx1a, nx1b = {}, {}
                ma, mb = ms[0:2], ms[2:4]
                cl.append((3500, None, 3, emit_ffn2,
                           (ma[0], qh, w2_, hTq, l, g1a, 0, 2)))
                cl.append((3500, None, 3, emit_ffn2,
                           (ma[1], qh, w2_, hTq, l, g1a, 1, 2)))
                if l < L - 1:
                    nTn = nTs[l + 1]
                    wqn, wkn, wvn = W[l + 1][0], W[l + 1][1], W[l + 1][2]
                    if qh == 0:
                        KV[l + 1] = alloc_kv()
                    cl.append((0, None, 3, emit_ln_finish, (ma, g1a, nx1a, nx1p)))
                    for m in ma:
                        cl.append((560, None, 4, emit_transp_m, (nx1a, nTn, m, 0)))
                    cl.append((3500, None, 4, emit_ffn2,
                               (mb[0], qh, w2_, hTq, l, g1b, 0, 2)))
                    cl.append((3500, None, 4, emit_ffn2,
                               (mb[1], qh, w2_, hTq, l, g1b, 1, 2)))
                    cl.append((0, None, 4, emit_ln_finish, (mb, g1b, nx1b, nx1p)))
                    cl += emit_kvq_pair(KV[l + 1], nTn, wqn, wkn, wvn, qh, 0, 5)
                    for m in mb:
                        cl.append((560, None, 5, emit_transp_m, (nx1b, nTn, m, 0)))
                    cl += emit_kvq_pair(KV[l + 1], nTn, wqn, wkn, wvn, qh, 1, 6)
                else:
                    cl.append((3500, None, 4, emit_ffn2,
                               (mb[0], qh, w2_, hTq, l, g1b, 0, 2)))
                    cl.append((3500, None, 4, emit_ffn2,
                               (mb[1], qh, w2_, hTq, l, g1b, 1, 2)))
                return cl

            def attn_loop(l, qh, pending):
                kv = KV[l]
                total = sum(c for c, k, s, f, a in pending)
                keyidx = {k: i for i, (c, k, s, f, a) in enumerate(pending) if k}
                state = {"done": 0, "spent": 0}

                def drain(need_idx, want_cost, stage_limit=False):
                    crossed = 0
                    while state["done"] < len(pending):
                        c, k, s, f, a = pending[state["done"]]
                        forced = state["done"] < need_idx
                        if not forced:
                            if state["spent"] >= want_cost:
                                break
                            if stage_limit and state["done"] > 0:
                                ps_ = pending[state["done"] - 1][2]
                                if s > ps_:
                                    crossed += 1
                                    if crossed > 1:
                                        break
                        f(*a)
                        state["spent"] += c
                        state["done"] += 1

                prev = None
                for quad in range(2):
                    cq = ps_cq.tile([128, 2, 512], F32, tag="cq", name="cq")
                    for mk in range(SM):
                        step = quad * SM + mk + 1   # 1..16
                        need = 0
                        if qh == 0:
                            if mk >= 4:
                                need = max(need, keyidx.get(
                                    ("kt", quad, (mk - 4) // 2), -1) + 1)
                            if prev is not None and prev[1] >= 4:
                                need = max(need, keyidx.get(("v", prev[1]), -1) + 1)
                        want = (total * max(0, step - 1)) // 24
                        drain(need, want, stage_limit=False)
                        pt = emit_scores_exp(kv, quad, mk, qh)
                        if prev is not None:
                            pq, pm, ppt, pcq = prev
                            emit_ctx(kv, pq, pm, ppt, pcq)
                            if pm == SM - 1:
                                emit_ctx_norm(pq, qh, pcq)
                        prev = (quad, mk, pt, cq)
                drain(len(pending), total)
                pq, pm, ppt, pcq = prev
                emit_ctx(kv, pq, pm, ppt, pcq)
                emit_ctx_norm(pq, qh, pcq)

            # boot: LN group 0 + transposes 0-3, then K/V/Q half-0
            # projections interleaved with group 1 + transposes 4-7.
            for l in range(L):
                nTs[l] = trs.tile([128, DC, S], BF16, tag="nT", name="nT")
            wq0, wk0, wv0 = W[0][0], W[0][1], W[0][2]
            KV[0] = alloc_kv()
            nxb = {}
            queue = []
            for pair in range(4):
                gb = {}
                for i in range(2):
                    emit_ln_stats(2 * pair + i, gb, i, 2)
                emit_ln_finish([2 * pair, 2 * pair + 1], gb, nxb, nx1p)
                emit_transp_m(nxb, nTs[0], 2 * pair, 0)
                emit_transp_m(nxb, nTs[0], 2 * pair + 1, 0)
                for q in queue:
                    q[3](*q[4])
                queue = []
                if pair == 0:
                    queue = emit_kvq_pair(KV[0], nTs[0], wq0, wk0, wv0, 0, 0, 0)
                elif pair == 1:
                    queue = emit_kvq_pair(KV[0], nTs[0], wq0, wk0, wv0, 0, 1, 0)
            for q in queue:
                q[3](*q[4])

            pending = (emit_kvq_pair(KV[0], nTs[0], wq0, wk0, wv0, 1, 0, 0)
                       + emit_kvq_pair(KV[0], nTs[0], wq0, wk0, wv0, 1, 1, 1))
            for l in range(L):
                attn_loop(l, 0, pending)
                pending = make_tail(l, 0)
                attn_loop(l, 1, pending)
                pending = make_tail(l, 1)
            # final tail (layer L-1 half 1) runs serially; its FFN2 closures
            # stream the output DMAs per token tile.
            for _, _, _, fn, args in pending:
                fn(*args)

    nc.compile()
    return nc


_CACHE = {}


def _get_module(with_mask):
    key = (with_mask,)
    if key not in _CACHE:
        _CACHE[key] = build_module(with_mask=with_mask)
    return _CACHE[key]


def _prep_weights(Wq, Wk, Wv, Wx, W1, W2):
    bf = ml_dtypes.bfloat16

    # Q/K: pad head columns from 12 to 32 (head h=4q+j at col 128q+32j)
    def pad_qk(w):  # [L, 512, 96] -> [L, DC, 128, 256]
        out = np.zeros((L, D, 256), np.float32)
        for h in range(H):
            q, j = divmod(h, 4)
            out[:, :, 128 * q + 32 * j:128 * q + 32 * j + DK] = (
                w[:, :, DK * h:DK * (h + 1)]
            )
        return np.ascontiguousarray(out.reshape(L, DC, 128, 256)).astype(bf)

    wq = pad_qk(np.asarray(Wq))
    wk = pad_qk(np.asarray(Wk))
    wv = np.ascontiguousarray(np.asarray(Wv).reshape(L, DC, 128, 256)).astype(bf)
    # Wx rows (h=4q+j, dd) -> [32j+dd, quad, :]
    wx = np.ascontiguousarray(
        np.asarray(Wx).reshape(L, 2, 4, 32, D).transpose(0, 2, 3, 1, 4)
        .reshape(L, 128, 2, D)
    ).astype(bf)
    w1 = np.ascontiguousarray(np.asarray(W1).reshape(L, DC, 128, FF)).astype(bf)
    w2 = np.ascontiguousarray(np.asarray(W2).reshape(L, FC, 128, D)).astype(bf)
    return dict(wq=wq, wk=wk, wv=wv, wx=wx, w1=w1, w2=w2)


def kernel(inputs, mask, Wq, bq, Wk, bk, Wv, bv, Wx, bx, W1, b1, W2, b2, gamma, beta):
    inputs = np.asarray(inputs, np.float32)
    mask = np.asarray(mask)
    for nm, b in (("bq", bq), ("bk", bk), ("bv", bv), ("bx", bx), ("b1", b1), ("b2", b2)):
        assert not np.any(np.asarray(b)), f"nonzero bias {nm} not supported"
    assert np.all(np.asarray(gamma) == 1.0) and not np.any(np.asarray(beta)), (
        "non-identity layernorm affine not supported"
    )

    with_mask = bool(np.any(np.asarray(mask) == 0))
    nc = _get_module(with_mask)
    wmap = _prep_weights(
        np.asarray(Wq, np.float32), np.asarray(Wk, np.float32),
        np.asarray(Wv, np.float32), np.asarray(Wx, np.float32),
        np.asarray(W1, np.float32), np.asarray(W2, np.float32),
    )

    in_maps = []
    for b in range(NCORES):
        m = dict(wmap)
        m["x"] = np.ascontiguousarray(inputs[b])
        if with_mask:
            m["maskf"] = np.ascontiguousarray((mask[b, 0] != 0).astype(np.float32))
        in_maps.append(m)

    import os
    from concourse.bass_utils import run_bass_kernel_spmd

    kw = {}
    tdir = os.environ.get("BASS_KERNEL_TRACE_DIR")
    if tdir:
        kw = dict(trace=True, tmpdir=tdir)
    res = run_bass_kernel_spmd(nc, in_maps, core_ids=list(range(NCORES)), **kw)
    global LAST_EXEC_NS
    LAST_EXEC_NS = res.exec_time_ns
    out = np.stack([res.results[i]["out"] for i in range(NCORES)], axis=0)
    return out.astype(np.float32)


LAST_EXEC_NS = None
